# revision 14
# baseline (speedup 1.0000x reference)
"""Multi-head attention (B=2, S=2048, D=1024, H=16) on 8 Trainium2 NeuronCores.

Sharding: data-parallel over batch (2) x tensor-parallel over head groups (4),
so each of the 8 cores owns 4 heads of one batch element: its own slice of the
QKV projections, its own 4x(2048x2048) score slabs, softmax, and attention
output columns. All outputs are disjoint -> no collectives.

Kernel structure per core:
  1. PE-transpose x (2048x1024) -> xT chunks [128 d, 2048 s] (fp32r).
  2. fp32r projections: qT/kT per head as [65, 2048] (w on partitions; row 64
     carries the additive-mask fold: qT row = 1.0, kT row = -1e4*(1-mask));
     wq is pre-scaled by 1/8 host-side so score PSUM = final scores.
     v as [128 s, 256 w] tiles (natural layout for the AV contraction).
  3. Pass A (q-major): scores matmuls -> PSUM [128,2048]; DVE copies scores
     out, ACT computes Exp with fused per-row accum (softmax denominator),
     DVE reciprocal, GPSIMD normalizes -> scores_soft. DMA both out.
  4. Pass B (k-major recompute): scoresT matmuls -> ACT Exp (fp32r) -> AV
     matmuls accumulate hT [64, 2048] in PSUM over all 16 k-tiles.
  5. PE-transpose hT, ACT copy scaled by 1/denominator -> h output.
"""

import os
import sys

if "/opt/trn_rl_repo" not in sys.path:
    sys.path.insert(0, "/opt/trn_rl_repo")

import numpy as np

B, S, D, H = 2, 2048, 1024, 16
W = 64            # head width
N_CORES = 8
HPC = 4           # heads per core
DPC = HPC * W     # 256 projection columns per core
NQ = S // 128     # 16 row tiles
ND = D // 128     # 8 contraction chunks

_compiled = None
last_results = None  # stashed BassKernelResults for test harnesses


def _build_program():
    import concourse.bass as bass
    import concourse.bacc as bacc
    import concourse.tile as tile
    import concourse.mybir as mybir

    F32R = mybir.dt.float32r
    F32 = mybir.dt.float32
    AF = mybir.ActivationFunctionType
    PSUM = bass.MemorySpace.PSUM

    nc = bacc.Bacc("TRN2", target_bir_lowering=False, debug=False,
                   num_devices=N_CORES)

    x_d = nc.dram_tensor("x", (S, D), F32R, kind="ExternalInput")
    wq_d = nc.dram_tensor("wq", (D, DPC), F32R, kind="ExternalInput")
    wk_d = nc.dram_tensor("wk", (D, DPC), F32R, kind="ExternalInput")
    wv_d = nc.dram_tensor("wv", (D, DPC), F32R, kind="ExternalInput")
    bq_d = nc.dram_tensor("bq", (1, DPC), F32R, kind="ExternalInput")
    bk_d = nc.dram_tensor("bk", (1, DPC), F32R, kind="ExternalInput")
    bv_d = nc.dram_tensor("bv", (1, DPC), F32R, kind="ExternalInput")
    mk_d = nc.dram_tensor("mk", (1, S), F32R, kind="ExternalInput")
    on_d = nc.dram_tensor("onesrow", (1, S), F32R, kind="ExternalInput")
    id_d = nc.dram_tensor("ident", (128, 128), F32R, kind="ExternalInput")

    sc_d = nc.dram_tensor("scores", (HPC * S, S), F32, kind="ExternalOutput")
    sm_d = nc.dram_tensor("soft", (HPC * S, S), F32, kind="ExternalOutput")
    h_d = nc.dram_tensor("hout", (S, DPC), F32, kind="ExternalOutput")

    with tile.TileContext(nc) as tc:
        with (
            tc.tile_pool(name="consts", bufs=1) as cp,
            tc.tile_pool(name="persist", bufs=1) as pp,
        ):
            ident = cp.tile([128, 128], F32R, tag="ident")
            nc.sync.dma_start(ident[:], id_d.ap()[:])
            ones = cp.tile([1, 512], F32R, tag="ones")
            nc.sync.dma_start(ones[:], on_d.ap()[0:1, 0:512])
            bq_sb = cp.tile([1, DPC], F32R, tag="bq")
            bk_sb = cp.tile([1, DPC], F32R, tag="bk")
            bv_sb = cp.tile([1, DPC], F32R, tag="bv")
            nc.sync.dma_start(bq_sb[:], bq_d.ap()[:])
            nc.sync.dma_start(bk_sb[:], bk_d.ap()[:])
            nc.sync.dma_start(bv_sb[:], bv_d.ap()[:])

            qT = [pp.tile([65, S], F32R, tag=f"qT{h}", name=f"qT{h}") for h in range(HPC)]
            kT = [pp.tile([65, S], F32R, tag=f"kT{h}", name=f"kT{h}") for h in range(HPC)]
            vv = [pp.tile([128, DPC], F32R, tag=f"v{st}", name=f"v{st}") for st in range(NQ)]
            dn = [pp.tile([128, NQ], F32, tag=f"dn{h}", name=f"dn{h}") for h in range(HPC)]
            rc = [pp.tile([128, NQ], F32, tag=f"rc{h}", name=f"rc{h}") for h in range(HPC)]
            for h in range(HPC):
                nc.sync.dma_start(qT[h][64:65, :], on_d.ap()[:])
                nc.sync.dma_start(kT[h][64:65, :], mk_d.ap()[:])

            # ---- Phase 1+2: transpose x, load weights, project QKV ----
            with (
                tc.tile_pool(name="xt", bufs=1) as xtp,
                tc.tile_pool(name="wts", bufs=1) as wtp,
            ):
                xT = [xtp.tile([128, S], F32R, tag=f"xT{dc}", name=f"xT{dc}") for dc in range(ND)]
                wq_sb = [wtp.tile([128, DPC], F32R, tag=f"wq{dc}", name=f"wq{dc}") for dc in range(ND)]
                wk_sb = [wtp.tile([128, DPC], F32R, tag=f"wk{dc}", name=f"wk{dc}") for dc in range(ND)]
                wv_sb = [wtp.tile([128, DPC], F32R, tag=f"wv{dc}", name=f"wv{dc}") for dc in range(ND)]
                with (
                    tc.tile_pool(name="ph1", bufs=4) as p1,
                    tc.tile_pool(name="ph1ps", bufs=4, space=PSUM) as p1ps,
                ):
                    for dc in range(ND):
                        sl = slice(dc * 128, (dc + 1) * 128)
                        nc.sync.dma_start(wq_sb[dc][:], wq_d.ap()[sl, :])
                        nc.sync.dma_start(wk_sb[dc][:], wk_d.ap()[sl, :])
                        nc.sync.dma_start(wv_sb[dc][:], wv_d.ap()[sl, :])
                    for dc in range(ND):
                        for sc in range(NQ):
                            xin = p1.tile([128, 128], F32R, tag="xin")
                            nc.sync.dma_start(
                                xin[:],
                                x_d.ap()[sc * 128:(sc + 1) * 128,
                                         dc * 128:(dc + 1) * 128])
                            pst = p1ps.tile([128, 128], F32R, tag="pst")
                            nc.tensor.transpose(pst[:], xin[:], ident[:])
                            eng = nc.vector if (dc * NQ + sc) % 2 == 0 else nc.scalar
                            if eng is nc.vector:
                                nc.vector.tensor_copy(
                                    xT[dc][:, sc * 128:(sc + 1) * 128], pst[:])
                            else:
                                nc.scalar.copy(
                                    xT[dc][:, sc * 128:(sc + 1) * 128], pst[:])

                with tc.tile_pool(name="ph2ps", bufs=2, space=PSUM) as p2ps:
                    # v: [128 s, 256 w] per s-tile
                    for st in range(NQ):
                        psv = p2ps.tile([128, DPC], F32, tag="psv")
                        for dc in range(ND):
                            nc.tensor.matmul(
                                psv[:],
                                xT[dc][:, st * 128:(st + 1) * 128],
                                wv_sb[dc][:],
                                start=(dc == 0), stop=False)
                        nc.tensor.matmul(
                            psv[:], ones[0:1, 0:128],
                            bv_sb[:], start=False, stop=True)
                        nc.vector.tensor_copy(vv[st][:], psv[:])
                    # qT/kT per head: [64 w, 2048 s]
                    for h in range(HPC):
                        hsl = slice(h * W, (h + 1) * W)
                        for sl4 in range(4):
                            ssl = slice(sl4 * 512, (sl4 + 1) * 512)
                            psq = p2ps.tile([64, 512], F32, tag="psq")
                            psk = p2ps.tile([64, 512], F32, tag="psk")
                            for dc in range(ND):
                                nc.tensor.matmul(
                                    psq[:], wq_sb[dc][:, hsl], xT[dc][:, ssl],
                                    start=(dc == 0), stop=False)
                            nc.tensor.matmul(
                                psq[:], bq_sb[0:1, hsl], ones[0:1, 0:512],
                                start=False, stop=True)
                            nc.scalar.copy(qT[h][0:64, ssl], psq[:])
                            for dc in range(ND):
                                nc.tensor.matmul(
                                    psk[:], wk_sb[dc][:, hsl], xT[dc][:, ssl],
                                    start=(dc == 0), stop=False)
                            nc.tensor.matmul(
                                psk[:], bk_sb[0:1, hsl], ones[0:1, 0:512],
                                start=False, stop=True)
                            nc.scalar.copy(kT[h][0:64, ssl], psk[:])

            # ---- Phase 3A: q-major scores, exp+denominator, soft ----
            with (
                tc.tile_pool(name="pA", bufs=2, space=PSUM) as pA,
                tc.tile_pool(name="oA", bufs=3) as oA,
            ):
                for h in range(HPC):
                    for i in range(NQ):
                        qsl = slice(i * 128, (i + 1) * 128)
                        ps = pA.tile([128, S], F32, tag="psA")
                        for j in range(4):
                            jsl = slice(j * 512, (j + 1) * 512)
                            nc.tensor.matmul(
                                ps[:, jsl], qT[h][0:65, qsl], kT[h][0:65, jsl],
                                start=True, stop=True)
                        scs = oA.tile([128, S], F32, tag="scs")
                        nc.vector.tensor_copy(scs[:], ps[:])
                        nc.sync.dma_start(
                            sc_d.ap()[h * S + i * 128:h * S + (i + 1) * 128, :],
                            scs[:])
                        ex = oA.tile([128, S], F32, tag="ex")
                        nc.scalar.activation(ex[:], ps[:], AF.Exp,
                                             accum_out=dn[h][:, i:i + 1])
                        nc.vector.reciprocal(rc[h][:, i:i + 1],
                                             dn[h][:, i:i + 1])
                        sf = oA.tile([128, S], F32, tag="sf")
                        nc.gpsimd.tensor_scalar_mul(sf[:], ex[:],
                                                    rc[h][:, i:i + 1])
                        nc.sync.dma_start(
                            sm_d.ap()[h * S + i * 128:h * S + (i + 1) * 128, :],
                            sf[:])

            # ---- Phase 3B: k-major recompute + AV accumulation ----
            with tc.tile_pool(name="htp", bufs=1) as hp:
              hT = [hp.tile([64, S], F32R, tag=f"hT{h}", name=f"hT{h}")
                    for h in range(HPC)]
              with (
                tc.tile_pool(name="pB", bufs=2, space=PSUM) as pB,
                tc.tile_pool(name="pH", bufs=1, space=PSUM) as pH,
                tc.tile_pool(name="oB", bufs=3) as oB,
              ):
                for h in range(HPC):
                    hsl = slice(h * W, (h + 1) * W)
                    psh = pH.tile([64, S], F32, tag="psh")
                    for j in range(NQ):
                        ksl = slice(j * 128, (j + 1) * 128)
                        for half in range(2):
                            psb = pB.tile([128, 1024], F32, tag="psb")
                            for jj in range(2):
                                q0 = (half * 2 + jj) * 512
                                nc.tensor.matmul(
                                    psb[:, jj * 512:(jj + 1) * 512],
                                    kT[h][0:65, ksl],
                                    qT[h][0:65, q0:q0 + 512],
                                    start=True, stop=True)
                            exb = oB.tile([128, 1024], F32R, tag="exb")
                            nc.scalar.activation(exb[:], psb[:], AF.Exp)
                            for jj in range(2):
                                q0 = (half * 2 + jj) * 512
                                nc.tensor.matmul(
                                    psh[:, q0:q0 + 512],
                                    vv[j][:, hsl],
                                    exb[:, jj * 512:(jj + 1) * 512],
                                    start=(j == 0), stop=(j == NQ - 1))
                    nc.vector.tensor_copy(hT[h][:], psh[:])

              # ---- Phase 3C: transpose hT, normalize, emit h ----
              with (
                tc.tile_pool(name="pT", bufs=4, space=PSUM) as pT,
                tc.tile_pool(name="oC", bufs=4) as oC,
              ):
                for h in range(HPC):
                    for i in range(NQ):
                        pst = pT.tile([128, 64], F32R, tag="pst3")
                        nc.tensor.transpose(
                            pst[:], hT[h][0:64, i * 128:(i + 1) * 128],
                            ident[0:64, 0:64])
                        ho = oC.tile([128, 64], F32, tag="ho")
                        nc.scalar.activation(ho[:], pst[:], AF.Copy,
                                             scale=rc[h][:, i:i + 1])
                        nc.sync.dma_start(
                            h_d.ap()[i * 128:(i + 1) * 128,
                                     h * W:(h + 1) * W],
                            ho[:])

    nc.compile()
    return nc


def kernel(**inputs) -> tuple:
    global _compiled, last_results
    from concourse import bass_utils

    x = np.asarray(inputs["x"], np.float32)
    mask = np.asarray(inputs["mask"], np.float32)
    wq = np.asarray(inputs["wq"], np.float32)
    bq = np.asarray(inputs["bq"], np.float32)
    wk = np.asarray(inputs["wk"], np.float32)
    bk = np.asarray(inputs["bk"], np.float32)
    wv = np.asarray(inputs["wv"], np.float32)
    bv = np.asarray(inputs["bv"], np.float32)

    if _compiled is None:
        _compiled = _build_program()
    nc = _compiled

    ident = np.eye(128, dtype=np.float32)
    in_maps = []
    for c in range(N_CORES):
        b, hg = divmod(c, HPC)
        dsl = slice(hg * DPC, (hg + 1) * DPC)
        in_maps.append({
            "x": np.ascontiguousarray(x[b]),
            "wq": np.ascontiguousarray(wq[:, dsl]) * 0.125,
            "wk": np.ascontiguousarray(wk[:, dsl]),
            "wv": np.ascontiguousarray(wv[:, dsl]),
            "bq": (bq[dsl] * 0.125).reshape(1, DPC).astype(np.float32),
            "bk": bk[dsl].reshape(1, DPC).astype(np.float32),
            "bv": bv[dsl].reshape(1, DPC).astype(np.float32),
            "mk": (-10000.0 * (1.0 - mask[b])).reshape(1, S).astype(np.float32),
            "onesrow": np.ones((1, S), np.float32),
            "ident": ident,
        })

    res = bass_utils.run_bass_kernel_spmd(
        nc, in_maps, core_ids=list(range(N_CORES)),
        trace=bool(os.environ.get("KERNEL_TRACE")))
    last_results = res

    h_full = np.empty((B, S, D), np.float32)
    scores = np.empty((B, H, S, S), np.float32)
    soft = np.empty((B, H, S, S), np.float32)
    for c in range(N_CORES):
        b, hg = divmod(c, HPC)
        r = res.results[c]
        scores[b, hg * HPC:(hg + 1) * HPC] = r["scores"].reshape(HPC, S, S)
        soft[b, hg * HPC:(hg + 1) * HPC] = r["soft"].reshape(HPC, S, S)
        h_full[b, :, hg * DPC:(hg + 1) * DPC] = r["hout"]
    return h_full, soft, scores


# revision 15
# speedup vs baseline: 2.8382x; 2.8382x over previous
"""Multi-head attention (B=2, S=2048, D=1024, H=16) on 8 Trainium2 NeuronCores.

Sharding: data-parallel over batch (2) x tensor-parallel over head groups (4),
so each of the 8 cores owns 4 heads of one batch element: its own slice of the
QKV projections, its own 4x(2048x2048) score slabs, softmax, and attention
output columns. All outputs are disjoint -> no collectives.

Kernel structure per core:
  1. PE-transpose x (2048x1024) -> xT chunks [128 d, 2048 s] (fp32r).
  2. fp32r projections: qT/kT per head as [65, 2048] (w on partitions; row 64
     carries the additive-mask fold: qT row = 1.0, kT row = -1e4*(1-mask));
     wq is pre-scaled by 1/8 host-side so score PSUM = final scores.
     v as [128 s, 256 w] tiles (natural layout for the AV contraction).
  3. Pass A (q-major): scores matmuls -> PSUM [128,2048]; DVE copies scores
     out, ACT computes Exp with fused per-row accum (softmax denominator),
     DVE reciprocal, GPSIMD normalizes -> scores_soft. DMA both out.
  4. Pass B (k-major recompute): scoresT matmuls -> ACT Exp (fp32r) -> AV
     matmuls accumulate hT [64, 2048] in PSUM over all 16 k-tiles.
  5. PE-transpose hT, ACT copy scaled by 1/denominator -> h output.
"""

import os
import sys

if "/opt/trn_rl_repo" not in sys.path:
    sys.path.insert(0, "/opt/trn_rl_repo")

import numpy as np

B, S, D, H = 2, 2048, 1024, 16
W = 64            # head width
N_CORES = 8
HPC = 4           # heads per core
DPC = HPC * W     # 256 projection columns per core
NQ = S // 128     # 16 row tiles
ND = D // 128     # 8 contraction chunks

_compiled = None
last_results = None  # stashed BassKernelResults for test harnesses


def _build_program():
    import concourse.bass as bass
    import concourse.bacc as bacc
    import concourse.tile as tile
    import concourse.mybir as mybir

    F32R = mybir.dt.float32r
    F32 = mybir.dt.float32
    AF = mybir.ActivationFunctionType
    PSUM = bass.MemorySpace.PSUM

    nc = bacc.Bacc("TRN2", target_bir_lowering=False, debug=False,
                   num_devices=N_CORES)

    x_d = nc.dram_tensor("x", (S, D), F32R, kind="ExternalInput")
    wq_d = nc.dram_tensor("wq", (D, DPC), F32R, kind="ExternalInput")
    wk_d = nc.dram_tensor("wk", (D, DPC), F32R, kind="ExternalInput")
    wv_d = nc.dram_tensor("wv", (D, DPC), F32R, kind="ExternalInput")
    bq_d = nc.dram_tensor("bq", (1, DPC), F32R, kind="ExternalInput")
    bk_d = nc.dram_tensor("bk", (1, DPC), F32R, kind="ExternalInput")
    bv_d = nc.dram_tensor("bv", (1, DPC), F32R, kind="ExternalInput")
    mk_d = nc.dram_tensor("mk", (1, S), F32R, kind="ExternalInput")
    on_d = nc.dram_tensor("onesrow", (1, S), F32R, kind="ExternalInput")
    id_d = nc.dram_tensor("ident", (128, 128), F32R, kind="ExternalInput")

    sc_d = nc.dram_tensor("scores", (HPC * S, S), F32, kind="ExternalOutput")
    sm_d = nc.dram_tensor("soft", (HPC * S, S), F32, kind="ExternalOutput")
    h_d = nc.dram_tensor("hout", (S, DPC), F32, kind="ExternalOutput")

    with tile.TileContext(nc) as tc:
        with (
            tc.tile_pool(name="consts", bufs=1) as cp,
            tc.tile_pool(name="persist", bufs=1) as pp,
        ):
            ident = cp.tile([128, 128], F32R, tag="ident")
            nc.sync.dma_start(ident[:], id_d.ap()[:])
            ones = cp.tile([1, 512], F32R, tag="ones")
            nc.sync.dma_start(ones[:], on_d.ap()[0:1, 0:512])
            bq_sb = cp.tile([1, DPC], F32R, tag="bq")
            bk_sb = cp.tile([1, DPC], F32R, tag="bk")
            bv_sb = cp.tile([1, DPC], F32R, tag="bv")
            nc.sync.dma_start(bq_sb[:], bq_d.ap()[:])
            nc.sync.dma_start(bk_sb[:], bk_d.ap()[:])
            nc.sync.dma_start(bv_sb[:], bv_d.ap()[:])

            qT = [pp.tile([65, S], F32R, tag=f"qT{h}", name=f"qT{h}") for h in range(HPC)]
            kT = [pp.tile([65, S], F32R, tag=f"kT{h}", name=f"kT{h}") for h in range(HPC)]
            vv = [pp.tile([128, DPC], F32R, tag=f"v{st}", name=f"v{st}") for st in range(NQ)]
            dn = [pp.tile([128, NQ], F32, tag=f"dn{h}", name=f"dn{h}") for h in range(HPC)]
            rc = [pp.tile([128, NQ], F32, tag=f"rc{h}", name=f"rc{h}") for h in range(HPC)]
            for h in range(HPC):
                nc.sync.dma_start(qT[h][64:65, :], on_d.ap()[:])
                nc.sync.dma_start(kT[h][64:65, :], mk_d.ap()[:])

            # ---- Phase 1+2: transpose x, load weights, project QKV ----
            with (
                tc.tile_pool(name="xt", bufs=1) as xtp,
                tc.tile_pool(name="wts", bufs=1) as wtp,
            ):
                xT = [xtp.tile([128, S], F32R, tag=f"xT{dc}", name=f"xT{dc}") for dc in range(ND)]
                wq_sb = [wtp.tile([128, DPC], F32R, tag=f"wq{dc}", name=f"wq{dc}") for dc in range(ND)]
                wk_sb = [wtp.tile([128, DPC], F32R, tag=f"wk{dc}", name=f"wk{dc}") for dc in range(ND)]
                wv_sb = [wtp.tile([128, DPC], F32R, tag=f"wv{dc}", name=f"wv{dc}") for dc in range(ND)]
                with (
                    tc.tile_pool(name="ph1", bufs=4) as p1,
                    tc.tile_pool(name="ph1ps", bufs=4, space=PSUM) as p1ps,
                ):
                    for dc in range(ND):
                        sl = slice(dc * 128, (dc + 1) * 128)
                        nc.sync.dma_start(wq_sb[dc][:], wq_d.ap()[sl, :])
                        nc.sync.dma_start(wk_sb[dc][:], wk_d.ap()[sl, :])
                        nc.sync.dma_start(wv_sb[dc][:], wv_d.ap()[sl, :])
                    for dc in range(ND):
                        for sc in range(NQ):
                            xin = p1.tile([128, 128], F32R, tag="xin")
                            nc.sync.dma_start(
                                xin[:],
                                x_d.ap()[sc * 128:(sc + 1) * 128,
                                         dc * 128:(dc + 1) * 128])
                            pst = p1ps.tile([128, 128], F32R, tag="pst")
                            nc.tensor.transpose(pst[:], xin[:], ident[:])
                            eng = nc.vector if (dc * NQ + sc) % 2 == 0 else nc.scalar
                            if eng is nc.vector:
                                nc.vector.tensor_copy(
                                    xT[dc][:, sc * 128:(sc + 1) * 128], pst[:])
                            else:
                                nc.scalar.copy(
                                    xT[dc][:, sc * 128:(sc + 1) * 128], pst[:])

                with tc.tile_pool(name="ph2ps", bufs=2, space=PSUM) as p2ps:
                    # v: [128 s, 256 w] per s-tile
                    for st in range(NQ):
                        psv = p2ps.tile([128, DPC], F32, tag="psv")
                        for dc in range(ND):
                            nc.tensor.matmul(
                                psv[:],
                                xT[dc][:, st * 128:(st + 1) * 128],
                                wv_sb[dc][:],
                                start=(dc == 0), stop=False)
                        nc.tensor.matmul(
                            psv[:], ones[0:1, 0:128],
                            bv_sb[:], start=False, stop=True)
                        nc.vector.tensor_copy(vv[st][:], psv[:])
                    # qT/kT per head: [64 w, 2048 s]
                    for h in range(HPC):
                        hsl = slice(h * W, (h + 1) * W)
                        for sl4 in range(4):
                            ssl = slice(sl4 * 512, (sl4 + 1) * 512)
                            psq = p2ps.tile([64, 512], F32, tag="psq")
                            psk = p2ps.tile([64, 512], F32, tag="psk")
                            for dc in range(ND):
                                nc.tensor.matmul(
                                    psq[:], wq_sb[dc][:, hsl], xT[dc][:, ssl],
                                    start=(dc == 0), stop=False)
                            nc.tensor.matmul(
                                psq[:], bq_sb[0:1, hsl], ones[0:1, 0:512],
                                start=False, stop=True)
                            nc.scalar.copy(qT[h][0:64, ssl], psq[:])
                            for dc in range(ND):
                                nc.tensor.matmul(
                                    psk[:], wk_sb[dc][:, hsl], xT[dc][:, ssl],
                                    start=(dc == 0), stop=False)
                            nc.tensor.matmul(
                                psk[:], bk_sb[0:1, hsl], ones[0:1, 0:512],
                                start=False, stop=True)
                            nc.scalar.copy(kT[h][0:64, ssl], psk[:])

            # ---- Phase 3A: q-major scores, exp+denominator, soft ----
            with (
                tc.tile_pool(name="pA", bufs=2, space=PSUM) as pA,
                tc.tile_pool(name="oA", bufs=3) as oA,
            ):
                for h in range(HPC):
                    for i in range(NQ):
                        qsl = slice(i * 128, (i + 1) * 128)
                        ps = pA.tile([128, S], F32, tag="psA")
                        for j in range(4):
                            jsl = slice(j * 512, (j + 1) * 512)
                            nc.tensor.matmul(
                                ps[:, jsl], qT[h][0:65, qsl], kT[h][0:65, jsl],
                                start=True, stop=True)
                        scs = oA.tile([128, S], F32, tag="scs")
                        nc.vector.tensor_copy(scs[:], ps[:])
                        nc.sync.dma_start(
                            sc_d.ap()[h * S + i * 128:h * S + (i + 1) * 128, :],
                            scs[:])
                        ex = oA.tile([128, S], F32, tag="ex")
                        nc.scalar.activation(ex[:], ps[:], AF.Exp,
                                             accum_out=dn[h][:, i:i + 1])
                        nc.vector.reciprocal(rc[h][:, i:i + 1],
                                             dn[h][:, i:i + 1])
                        sf = oA.tile([128, S], F32, tag="sf")
                        nc.vector.tensor_scalar_mul(sf[:], ex[:],
                                                    rc[h][:, i:i + 1])
                        nc.sync.dma_start(
                            sm_d.ap()[h * S + i * 128:h * S + (i + 1) * 128, :],
                            sf[:])

            # ---- Phase 3B: k-major recompute + AV accumulation ----
            with tc.tile_pool(name="htp", bufs=1) as hp:
              hT = [hp.tile([64, S], F32R, tag=f"hT{h}", name=f"hT{h}")
                    for h in range(HPC)]
              with (
                tc.tile_pool(name="pB", bufs=2, space=PSUM) as pB,
                tc.tile_pool(name="pH", bufs=1, space=PSUM) as pH,
                tc.tile_pool(name="oB", bufs=3) as oB,
              ):
                for h in range(HPC):
                    hsl = slice(h * W, (h + 1) * W)
                    psh = pH.tile([64, S], F32, tag="psh")
                    for j in range(NQ):
                        ksl = slice(j * 128, (j + 1) * 128)
                        for half in range(2):
                            psb = pB.tile([128, 1024], F32, tag="psb")
                            for jj in range(2):
                                q0 = (half * 2 + jj) * 512
                                nc.tensor.matmul(
                                    psb[:, jj * 512:(jj + 1) * 512],
                                    kT[h][0:65, ksl],
                                    qT[h][0:65, q0:q0 + 512],
                                    start=True, stop=True)
                            exb = oB.tile([128, 1024], F32R, tag="exb")
                            nc.scalar.activation(exb[:], psb[:], AF.Exp)
                            for jj in range(2):
                                q0 = (half * 2 + jj) * 512
                                nc.tensor.matmul(
                                    psh[:, q0:q0 + 512],
                                    vv[j][:, hsl],
                                    exb[:, jj * 512:(jj + 1) * 512],
                                    start=(j == 0), stop=(j == NQ - 1))
                    nc.vector.tensor_copy(hT[h][:], psh[:])

              # ---- Phase 3C: transpose hT, normalize, emit h ----
              with (
                tc.tile_pool(name="pT", bufs=4, space=PSUM) as pT,
                tc.tile_pool(name="oC", bufs=4) as oC,
              ):
                for h in range(HPC):
                    for i in range(NQ):
                        pst = pT.tile([128, 64], F32R, tag="pst3")
                        nc.tensor.transpose(
                            pst[:], hT[h][0:64, i * 128:(i + 1) * 128],
                            ident[0:64, 0:64])
                        ho = oC.tile([128, 64], F32, tag="ho")
                        nc.scalar.activation(ho[:], pst[:], AF.Copy,
                                             scale=rc[h][:, i:i + 1])
                        nc.sync.dma_start(
                            h_d.ap()[i * 128:(i + 1) * 128,
                                     h * W:(h + 1) * W],
                            ho[:])

    nc.compile()
    return nc


def kernel(**inputs) -> tuple:
    global _compiled, last_results
    from concourse import bass_utils

    x = np.asarray(inputs["x"], np.float32)
    mask = np.asarray(inputs["mask"], np.float32)
    wq = np.asarray(inputs["wq"], np.float32)
    bq = np.asarray(inputs["bq"], np.float32)
    wk = np.asarray(inputs["wk"], np.float32)
    bk = np.asarray(inputs["bk"], np.float32)
    wv = np.asarray(inputs["wv"], np.float32)
    bv = np.asarray(inputs["bv"], np.float32)

    if _compiled is None:
        _compiled = _build_program()
    nc = _compiled

    ident = np.eye(128, dtype=np.float32)
    in_maps = []
    for c in range(N_CORES):
        b, hg = divmod(c, HPC)
        dsl = slice(hg * DPC, (hg + 1) * DPC)
        in_maps.append({
            "x": np.ascontiguousarray(x[b]),
            "wq": np.ascontiguousarray(wq[:, dsl]) * 0.125,
            "wk": np.ascontiguousarray(wk[:, dsl]),
            "wv": np.ascontiguousarray(wv[:, dsl]),
            "bq": (bq[dsl] * 0.125).reshape(1, DPC).astype(np.float32),
            "bk": bk[dsl].reshape(1, DPC).astype(np.float32),
            "bv": bv[dsl].reshape(1, DPC).astype(np.float32),
            "mk": (-10000.0 * (1.0 - mask[b])).reshape(1, S).astype(np.float32),
            "onesrow": np.ones((1, S), np.float32),
            "ident": ident,
        })

    res = bass_utils.run_bass_kernel_spmd(
        nc, in_maps, core_ids=list(range(N_CORES)),
        trace=bool(os.environ.get("KERNEL_TRACE")))
    last_results = res

    h_full = np.empty((B, S, D), np.float32)
    scores = np.empty((B, H, S, S), np.float32)
    soft = np.empty((B, H, S, S), np.float32)
    for c in range(N_CORES):
        b, hg = divmod(c, HPC)
        r = res.results[c]
        scores[b, hg * HPC:(hg + 1) * HPC] = r["scores"].reshape(HPC, S, S)
        soft[b, hg * HPC:(hg + 1) * HPC] = r["soft"].reshape(HPC, S, S)
        h_full[b, :, hg * DPC:(hg + 1) * DPC] = r["hout"]
    return h_full, soft, scores


# revision 16
# speedup vs baseline: 2.8943x; 1.0198x over previous
"""Multi-head attention (B=2, S=2048, D=1024, H=16) on 8 Trainium2 NeuronCores.

Sharding: data-parallel over batch (2) x tensor-parallel over head groups (4),
so each of the 8 cores owns 4 heads of one batch element: its own slice of the
QKV projections, its own 4x(2048x2048) score slabs, softmax, and attention
output columns. All outputs are disjoint -> no collectives.

Kernel structure per core (all matmuls fp32r = full-rate ~tf32):
  1. PE-transpose x (2048x1024) -> xT chunks [128 d, 2048 s].
  2. Projections: qT/kT per head as [67, 2048] (head width on partitions).
     Aug rows fold affine terms into the score matmuls:
       row 64: qT=1, kT=-1e4*(1-mask)  -> additive mask
       row 65/66: qT=-ln(denom) hi/lo, kT=1 -> softmax normalization (pass B
       only; written after pass A computes the denominators)
     wq is pre-scaled by 1/8 host-side so score PSUM = final scores.
     v as [128 s, 256 w] tiles (natural layout for the AV contraction).
  3. Per head, pass A (q-major, contraction rows 0..64): scores -> PSUM
     [128,1024] halves; DVE copies scores out; ACT Exp with fused per-row
     accum -> denominators; DVE reciprocal + normalize -> scores_soft.
  4. ln-prep: ACT Ln(1/denom) -> hi/lo fp32r split -> PE transpose ->
     DMA into qT rows 65/66.
  5. Pass B (k-major, contraction rows 0..66): scoresT+mask-ln(denom) ->
     ACT Exp (= normalized weights) -> AV matmuls accumulate hT [64,1024]
     per q-half -> copy out -> DMA. h is emitted transposed (DPC, S);
     the host transposes back.
"""

import os
import sys

if "/opt/trn_rl_repo" not in sys.path:
    sys.path.insert(0, "/opt/trn_rl_repo")

import numpy as np

B, S, D, H = 2, 2048, 1024, 16
W = 64            # head width
N_CORES = 8
HPC = 4           # heads per core
DPC = HPC * W     # 256 projection columns per core
NQ = S // 128     # 16 row tiles
ND = D // 128     # 8 contraction chunks

_compiled = None
last_results = None  # stashed BassKernelResults for test harnesses


def _build_program():
    import concourse.bass as bass
    import concourse.bacc as bacc
    import concourse.tile as tile
    import concourse.mybir as mybir

    F32R = mybir.dt.float32r
    F32 = mybir.dt.float32
    AF = mybir.ActivationFunctionType
    PSUM = bass.MemorySpace.PSUM

    nc = bacc.Bacc("TRN2", target_bir_lowering=False, debug=False,
                   num_devices=N_CORES)

    x_d = nc.dram_tensor("x", (S, D), F32R, kind="ExternalInput")
    wq_d = nc.dram_tensor("wq", (D, DPC), F32R, kind="ExternalInput")
    wk_d = nc.dram_tensor("wk", (D, DPC), F32R, kind="ExternalInput")
    wv_d = nc.dram_tensor("wv", (D, DPC), F32R, kind="ExternalInput")
    bq_d = nc.dram_tensor("bq", (1, DPC), F32R, kind="ExternalInput")
    bk_d = nc.dram_tensor("bk", (1, DPC), F32R, kind="ExternalInput")
    bv_d = nc.dram_tensor("bv", (1, DPC), F32R, kind="ExternalInput")
    mk_d = nc.dram_tensor("mk", (1, S), F32R, kind="ExternalInput")
    on_d = nc.dram_tensor("onesrow", (1, S), F32R, kind="ExternalInput")
    id_d = nc.dram_tensor("ident", (128, 128), F32R, kind="ExternalInput")

    sc_d = nc.dram_tensor("scores", (HPC * S, S), F32, kind="ExternalOutput")
    sm_d = nc.dram_tensor("soft", (HPC * S, S), F32, kind="ExternalOutput")
    hT_d = nc.dram_tensor("houtT", (DPC, S), F32, kind="ExternalOutput")

    with tile.TileContext(nc) as tc:
        with (
            tc.tile_pool(name="consts", bufs=1) as cp,
            tc.tile_pool(name="persist", bufs=1) as pp,
        ):
            ident = cp.tile([128, 128], F32R, tag="ident")
            nc.sync.dma_start(ident[:], id_d.ap()[:])
            ones = cp.tile([1, 512], F32R, tag="ones")
            nc.sync.dma_start(ones[:], on_d.ap()[0:1, 0:512])
            bq_sb = cp.tile([1, DPC], F32R, tag="bq")
            bk_sb = cp.tile([1, DPC], F32R, tag="bk")
            bv_sb = cp.tile([1, DPC], F32R, tag="bv")
            nc.sync.dma_start(bq_sb[:], bq_d.ap()[:])
            nc.sync.dma_start(bk_sb[:], bk_d.ap()[:])
            nc.sync.dma_start(bv_sb[:], bv_d.ap()[:])

            qT = [pp.tile([67, S], F32R, tag=f"qT{h}", name=f"qT{h}")
                  for h in range(HPC)]
            kT = [pp.tile([67, S], F32R, tag=f"kT{h}", name=f"kT{h}")
                  for h in range(HPC)]
            vv = [pp.tile([128, DPC], F32R, tag=f"v{st}", name=f"v{st}")
                  for st in range(NQ)]
            # per-head softmax stats: dn2 = per-half-row sums, ds = full sums,
            # rc = reciprocals
            dn2 = [pp.tile([128, 2 * NQ], F32, tag=f"dn{h}", name=f"dn{h}")
                   for h in range(HPC)]
            ds = [pp.tile([128, NQ], F32, tag=f"ds{h}", name=f"ds{h}")
                  for h in range(HPC)]
            rc = [pp.tile([128, NQ], F32, tag=f"rc{h}", name=f"rc{h}")
                  for h in range(HPC)]
            for h in range(HPC):
                nc.sync.dma_start(qT[h][64:65, :], on_d.ap()[:])
                nc.sync.dma_start(kT[h][64:65, :], mk_d.ap()[:])
                nc.sync.dma_start(kT[h][65:66, :], on_d.ap()[:])
                nc.sync.dma_start(kT[h][66:67, :], on_d.ap()[:])

            # ---- Phase 1+2: transpose x, load weights, project QKV ----
            with (
                tc.tile_pool(name="xt", bufs=1) as xtp,
                tc.tile_pool(name="wts", bufs=1) as wtp,
            ):
                xT = [xtp.tile([128, S], F32R, tag=f"xT{dc}", name=f"xT{dc}")
                      for dc in range(ND)]
                wq_sb = [wtp.tile([128, DPC], F32R, tag=f"wq{dc}", name=f"wq{dc}")
                         for dc in range(ND)]
                wk_sb = [wtp.tile([128, DPC], F32R, tag=f"wk{dc}", name=f"wk{dc}")
                         for dc in range(ND)]
                wv_sb = [wtp.tile([128, DPC], F32R, tag=f"wv{dc}", name=f"wv{dc}")
                         for dc in range(ND)]
                with (
                    tc.tile_pool(name="ph1", bufs=3) as p1,
                    tc.tile_pool(name="ph1ps", bufs=4, space=PSUM) as p1ps,
                ):
                    for dc in range(ND):
                        sl = slice(dc * 128, (dc + 1) * 128)
                        nc.sync.dma_start(wq_sb[dc][:], wq_d.ap()[sl, :])
                        nc.sync.dma_start(wk_sb[dc][:], wk_d.ap()[sl, :])
                        nc.sync.dma_start(wv_sb[dc][:], wv_d.ap()[sl, :])
                    for sc in range(NQ):
                        xin = p1.tile([128, D], F32R, tag="xin")
                        nc.sync.dma_start(
                            xin[:], x_d.ap()[sc * 128:(sc + 1) * 128, :])
                        for dc in range(ND):
                            pst = p1ps.tile([128, 128], F32R, tag="pst")
                            nc.tensor.transpose(
                                pst[:], xin[:, dc * 128:(dc + 1) * 128],
                                ident[:])
                            if (sc * ND + dc) % 2 == 0:
                                nc.vector.tensor_copy(
                                    xT[dc][:, sc * 128:(sc + 1) * 128], pst[:])
                            else:
                                nc.scalar.copy(
                                    xT[dc][:, sc * 128:(sc + 1) * 128], pst[:])

                with tc.tile_pool(name="ph2ps", bufs=2, space=PSUM) as p2ps:
                    # v: [128 s, 256 w] per s-tile
                    for st in range(NQ):
                        psv = p2ps.tile([128, DPC], F32, tag="psv")
                        for dc in range(ND):
                            nc.tensor.matmul(
                                psv[:],
                                xT[dc][:, st * 128:(st + 1) * 128],
                                wv_sb[dc][:],
                                start=(dc == 0), stop=False)
                        nc.tensor.matmul(
                            psv[:], ones[0:1, 0:128],
                            bv_sb[:], start=False, stop=True)
                        nc.vector.tensor_copy(vv[st][:], psv[:])
                    # qT/kT per head: [64 w, 2048 s]
                    for h in range(HPC):
                        hsl = slice(h * W, (h + 1) * W)
                        for sl4 in range(4):
                            ssl = slice(sl4 * 512, (sl4 + 1) * 512)
                            psq = p2ps.tile([64, 512], F32, tag="psq")
                            psk = p2ps.tile([64, 512], F32, tag="psk")
                            for dc in range(ND):
                                nc.tensor.matmul(
                                    psq[:], wq_sb[dc][:, hsl], xT[dc][:, ssl],
                                    start=(dc == 0), stop=False)
                            nc.tensor.matmul(
                                psq[:], bq_sb[0:1, hsl], ones[0:1, 0:512],
                                start=False, stop=True)
                            nc.scalar.copy(qT[h][0:64, ssl], psq[:])
                            for dc in range(ND):
                                nc.tensor.matmul(
                                    psk[:], wk_sb[dc][:, hsl], xT[dc][:, ssl],
                                    start=(dc == 0), stop=False)
                            nc.tensor.matmul(
                                psk[:], bk_sb[0:1, hsl], ones[0:1, 0:512],
                                start=False, stop=True)
                            nc.scalar.copy(kT[h][0:64, ssl], psk[:])

            # ---- Phase 3: per head: pass A, ln-prep, pass B ----
            with (
                tc.tile_pool(name="pA", bufs=2, space=PSUM) as pA,
                tc.tile_pool(name="pB", bufs=1, space=PSUM) as pB,
                tc.tile_pool(name="pH", bufs=1, space=PSUM) as pH,
                tc.tile_pool(name="oA", bufs=3) as oA,
                tc.tile_pool(name="oB", bufs=3) as oB,
                tc.tile_pool(name="lnp", bufs=2) as lnp,
            ):
                for h in range(HPC):
                    hsl = slice(h * W, (h + 1) * W)
                    # ---- pass A ----
                    for i in range(NQ):
                        qsl = slice(i * 128, (i + 1) * 128)
                        scs = oA.tile([128, S], F32, tag="scs")
                        ex = oA.tile([128, S], F32, tag="ex")
                        for half in range(2):
                            ps = pA.tile([128, 1024], F32, tag="psA")
                            for jj in range(2):
                                k0 = half * 1024 + jj * 512
                                nc.tensor.matmul(
                                    ps[:, jj * 512:(jj + 1) * 512],
                                    qT[h][0:65, qsl],
                                    kT[h][0:65, k0:k0 + 512],
                                    start=True, stop=True)
                            hs = slice(half * 1024, (half + 1) * 1024)
                            nc.vector.tensor_copy(scs[:, hs], ps[:])
                            nc.scalar.activation(
                                ex[:, hs], ps[:], AF.Exp,
                                accum_out=dn2[h][:, 2 * i + half:
                                                 2 * i + half + 1])
                        nc.sync.dma_start(
                            sc_d.ap()[h * S + i * 128:h * S + (i + 1) * 128, :],
                            scs[:])
                        nc.vector.tensor_add(ds[h][:, i:i + 1],
                                             dn2[h][:, 2 * i:2 * i + 1],
                                             dn2[h][:, 2 * i + 1:2 * i + 2])
                        nc.vector.reciprocal(rc[h][:, i:i + 1],
                                             ds[h][:, i:i + 1])
                        sf = oA.tile([128, S], F32, tag="sf")
                        nc.vector.tensor_scalar_mul(sf[:], ex[:],
                                                    rc[h][:, i:i + 1])
                        nc.sync.dma_start(
                            sm_d.ap()[h * S + i * 128:h * S + (i + 1) * 128, :],
                            sf[:])
                    # ---- ln-prep: qT rows 65/66 = -ln(denom) in hi+lo ----
                    lnq = lnp.tile([128, NQ], F32, tag="lnq")
                    nc.scalar.activation(lnq[:], rc[h][:], AF.Ln)
                    lnhi = lnp.tile([128, 2 * NQ], F32R, tag="lnhi")
                    nc.vector.tensor_copy(lnhi[:, 0:NQ], lnq[:])
                    nc.vector.tensor_sub(lnhi[:, NQ:2 * NQ], lnq[:],
                                         lnhi[:, 0:NQ])
                    pst2 = pB.tile([2 * NQ, 128], F32R, tag="psb")
                    nc.tensor.transpose(pst2[:], lnhi[:], ident[:])
                    lnrow = lnp.tile([2 * NQ, 128], F32R, tag="lnrow")
                    nc.vector.tensor_copy(lnrow[:], pst2[:])
                    nc.sync.dma_start(qT[h][65:66, :], lnrow[0:NQ, :])
                    nc.sync.dma_start(qT[h][66:67, :], lnrow[NQ:2 * NQ, :])
                    # ---- pass B + AV ----
                    for qhalf in range(2):
                        psh = pH.tile([64, 1024], F32, tag="psh")
                        for j in range(NQ):
                            ksl = slice(j * 128, (j + 1) * 128)
                            psb = pB.tile([128, 1024], F32, tag="psb")
                            for jj in range(2):
                                q0 = qhalf * 1024 + jj * 512
                                nc.tensor.matmul(
                                    psb[:, jj * 512:(jj + 1) * 512],
                                    kT[h][0:67, ksl],
                                    qT[h][0:67, q0:q0 + 512],
                                    start=True, stop=True)
                            exb = oB.tile([128, 1024], F32R, tag="exb")
                            nc.scalar.activation(exb[:], psb[:], AF.Exp)
                            for jj in range(2):
                                nc.tensor.matmul(
                                    psh[:, jj * 512:(jj + 1) * 512],
                                    vv[j][:, hsl],
                                    exb[:, jj * 512:(jj + 1) * 512],
                                    start=(j == 0), stop=(j == NQ - 1))
                        hst = oB.tile([64, 1024], F32, tag="hst")
                        nc.vector.tensor_copy(hst[:], psh[:])
                        nc.sync.dma_start(
                            hT_d.ap()[h * W:(h + 1) * W,
                                      qhalf * 1024:(qhalf + 1) * 1024],
                            hst[:])

    nc.compile()
    return nc


def kernel(**inputs) -> tuple:
    global _compiled, last_results
    from concourse import bass_utils

    x = np.asarray(inputs["x"], np.float32)
    mask = np.asarray(inputs["mask"], np.float32)
    wq = np.asarray(inputs["wq"], np.float32)
    bq = np.asarray(inputs["bq"], np.float32)
    wk = np.asarray(inputs["wk"], np.float32)
    bk = np.asarray(inputs["bk"], np.float32)
    wv = np.asarray(inputs["wv"], np.float32)
    bv = np.asarray(inputs["bv"], np.float32)

    if _compiled is None:
        _compiled = _build_program()
    nc = _compiled

    ident = np.eye(128, dtype=np.float32)
    in_maps = []
    for c in range(N_CORES):
        b, hg = divmod(c, HPC)
        dsl = slice(hg * DPC, (hg + 1) * DPC)
        in_maps.append({
            "x": np.ascontiguousarray(x[b]),
            "wq": np.ascontiguousarray(wq[:, dsl]) * 0.125,
            "wk": np.ascontiguousarray(wk[:, dsl]),
            "wv": np.ascontiguousarray(wv[:, dsl]),
            "bq": (bq[dsl] * 0.125).reshape(1, DPC).astype(np.float32),
            "bk": bk[dsl].reshape(1, DPC).astype(np.float32),
            "bv": bv[dsl].reshape(1, DPC).astype(np.float32),
            "mk": (-10000.0 * (1.0 - mask[b])).reshape(1, S).astype(np.float32),
            "onesrow": np.ones((1, S), np.float32),
            "ident": ident,
        })

    res = bass_utils.run_bass_kernel_spmd(
        nc, in_maps, core_ids=list(range(N_CORES)),
        trace=bool(os.environ.get("KERNEL_TRACE")))
    last_results = res

    h_full = np.empty((B, S, D), np.float32)
    scores = np.empty((B, H, S, S), np.float32)
    soft = np.empty((B, H, S, S), np.float32)
    for c in range(N_CORES):
        b, hg = divmod(c, HPC)
        r = res.results[c]
        scores[b, hg * HPC:(hg + 1) * HPC] = r["scores"].reshape(HPC, S, S)
        soft[b, hg * HPC:(hg + 1) * HPC] = r["soft"].reshape(HPC, S, S)
        h_full[b, :, hg * DPC:(hg + 1) * DPC] = r["houtT"].T
    return h_full, soft, scores


# revision 17
# speedup vs baseline: 3.0726x; 1.0616x over previous
"""Multi-head attention (B=2, S=2048, D=1024, H=16) on 8 Trainium2 NeuronCores.

Sharding: data-parallel over batch (2) x tensor-parallel over head groups (4),
so each of the 8 cores owns 4 heads of one batch element: its own slice of the
QKV projections, its own 4x(2048x2048) score slabs, softmax, and attention
output columns. All outputs are disjoint -> no collectives.

Kernel structure per core (all matmuls fp32r = full-rate ~tf32):
  1. PE-transpose x (2048x1024) -> xT chunks [128 d, 2048 s].
  2. Projections: qT/kT per head as [67, 2048] (head width on partitions).
     Aug rows fold affine terms into the score matmuls:
       row 64: qT=1, kT=-1e4*(1-mask)  -> additive mask
       row 65/66: qT=-ln(denom) hi/lo, kT=1 -> softmax normalization (pass B
       only; written after pass A computes the denominators)
     wq is pre-scaled by 1/8 host-side so score PSUM = final scores.
     v as [128 s, 256 w] tiles (natural layout for the AV contraction).
  3. Per head, pass A (q-major, contraction rows 0..64): scores -> PSUM
     [128,1024] halves; DVE copies scores out; ACT Exp with fused per-row
     accum -> denominators; DVE reciprocal + normalize -> scores_soft.
  4. ln-prep: ACT Ln(1/denom) -> hi/lo fp32r split -> PE transpose ->
     DMA into qT rows 65/66.
  5. Pass B (k-major, contraction rows 0..66): scoresT+mask-ln(denom) ->
     ACT Exp (= normalized weights) -> AV matmuls accumulate hT [64,1024]
     per q-half -> copy out -> DMA. h is emitted transposed (DPC, S);
     the host transposes back.
"""

import os
import sys

if "/opt/trn_rl_repo" not in sys.path:
    sys.path.insert(0, "/opt/trn_rl_repo")

import numpy as np

B, S, D, H = 2, 2048, 1024, 16
W = 64            # head width
N_CORES = 8
HPC = 4           # heads per core
DPC = HPC * W     # 256 projection columns per core
NQ = S // 128     # 16 row tiles
ND = D // 128     # 8 contraction chunks

_compiled = None
last_results = None  # stashed BassKernelResults for test harnesses


def _build_program():
    import concourse.bass as bass
    import concourse.bacc as bacc
    import concourse.tile as tile
    import concourse.mybir as mybir

    F32R = mybir.dt.float32r
    F32 = mybir.dt.float32
    AF = mybir.ActivationFunctionType
    PSUM = bass.MemorySpace.PSUM

    nc = bacc.Bacc("TRN2", target_bir_lowering=False, debug=False,
                   num_devices=N_CORES)

    x_d = nc.dram_tensor("x", (S, D), F32R, kind="ExternalInput")
    wq_d = nc.dram_tensor("wq", (D, DPC), F32R, kind="ExternalInput")
    wk_d = nc.dram_tensor("wk", (D, DPC), F32R, kind="ExternalInput")
    wv_d = nc.dram_tensor("wv", (D, DPC), F32R, kind="ExternalInput")
    bq_d = nc.dram_tensor("bq", (1, DPC), F32R, kind="ExternalInput")
    bk_d = nc.dram_tensor("bk", (1, DPC), F32R, kind="ExternalInput")
    bv_d = nc.dram_tensor("bv", (1, DPC), F32R, kind="ExternalInput")
    mk_d = nc.dram_tensor("mk", (1, S), F32R, kind="ExternalInput")
    on_d = nc.dram_tensor("onesrow", (1, S), F32R, kind="ExternalInput")
    id_d = nc.dram_tensor("ident", (128, 128), F32R, kind="ExternalInput")

    sc_d = nc.dram_tensor("scores", (HPC * S, S), F32, kind="ExternalOutput")
    sm_d = nc.dram_tensor("soft", (HPC * S, S), F32, kind="ExternalOutput")
    hT_d = nc.dram_tensor("houtT", (DPC, S), F32, kind="ExternalOutput")

    with tile.TileContext(nc) as tc:
        with (
            tc.tile_pool(name="consts", bufs=1) as cp,
            tc.tile_pool(name="persist", bufs=1) as pp,
        ):
            ident = cp.tile([128, 128], F32R, tag="ident")
            nc.sync.dma_start(ident[:], id_d.ap()[:])
            ones = cp.tile([1, 512], F32R, tag="ones")
            nc.sync.dma_start(ones[:], on_d.ap()[0:1, 0:512])
            bq_sb = cp.tile([1, DPC], F32R, tag="bq")
            bk_sb = cp.tile([1, DPC], F32R, tag="bk")
            bv_sb = cp.tile([1, DPC], F32R, tag="bv")
            nc.sync.dma_start(bq_sb[:], bq_d.ap()[:])
            nc.sync.dma_start(bk_sb[:], bk_d.ap()[:])
            nc.sync.dma_start(bv_sb[:], bv_d.ap()[:])

            qT = [pp.tile([67, S], F32R, tag=f"qT{h}", name=f"qT{h}")
                  for h in range(HPC)]
            kT = [pp.tile([67, S], F32R, tag=f"kT{h}", name=f"kT{h}")
                  for h in range(HPC)]
            vv = [pp.tile([128, DPC], F32R, tag=f"v{st}", name=f"v{st}")
                  for st in range(NQ)]
            # per-head softmax stats: dn2 = per-half-row sums, ds = full sums,
            # rc = reciprocals
            dn2 = [pp.tile([128, 2 * NQ], F32, tag=f"dn{h}", name=f"dn{h}")
                   for h in range(HPC)]
            ds = [pp.tile([128, NQ], F32, tag=f"ds{h}", name=f"ds{h}")
                  for h in range(HPC)]
            rc = [pp.tile([128, NQ], F32, tag=f"rc{h}", name=f"rc{h}")
                  for h in range(HPC)]
            for h in range(HPC):
                nc.sync.dma_start(qT[h][64:65, :], on_d.ap()[:])
                nc.sync.dma_start(kT[h][64:65, :], mk_d.ap()[:])
                nc.sync.dma_start(kT[h][65:66, :], on_d.ap()[:])
                nc.sync.dma_start(kT[h][66:67, :], on_d.ap()[:])

            # ---- Phase 1+2: transpose x, load weights, project QKV ----
            with (
                tc.tile_pool(name="xt", bufs=1) as xtp,
                tc.tile_pool(name="wts", bufs=1) as wtp,
            ):
                xT = [xtp.tile([128, S], F32R, tag=f"xT{dc}", name=f"xT{dc}")
                      for dc in range(ND)]
                wq_sb = [wtp.tile([128, DPC], F32R, tag=f"wq{dc}", name=f"wq{dc}")
                         for dc in range(ND)]
                wk_sb = [wtp.tile([128, DPC], F32R, tag=f"wk{dc}", name=f"wk{dc}")
                         for dc in range(ND)]
                wv_sb = [wtp.tile([128, DPC], F32R, tag=f"wv{dc}", name=f"wv{dc}")
                         for dc in range(ND)]
                with (
                    tc.tile_pool(name="ph1", bufs=3) as p1,
                    tc.tile_pool(name="ph1ps", bufs=4, space=PSUM) as p1ps,
                ):
                    for dc in range(ND):
                        sl = slice(dc * 128, (dc + 1) * 128)
                        nc.sync.dma_start(wq_sb[dc][:], wq_d.ap()[sl, :])
                        nc.sync.dma_start(wk_sb[dc][:], wk_d.ap()[sl, :])
                        nc.sync.dma_start(wv_sb[dc][:], wv_d.ap()[sl, :])
                    for sc in range(NQ):
                        xin = p1.tile([128, D], F32R, tag="xin")
                        nc.sync.dma_start(
                            xin[:], x_d.ap()[sc * 128:(sc + 1) * 128, :])
                        for dc in range(ND):
                            pst = p1ps.tile([128, 128], F32R, tag="pst")
                            nc.tensor.transpose(
                                pst[:], xin[:, dc * 128:(dc + 1) * 128],
                                ident[:])
                            if (sc * ND + dc) % 2 == 0:
                                nc.vector.tensor_copy(
                                    xT[dc][:, sc * 128:(sc + 1) * 128], pst[:])
                            else:
                                nc.scalar.copy(
                                    xT[dc][:, sc * 128:(sc + 1) * 128], pst[:])

                with tc.tile_pool(name="ph2ps", bufs=2, space=PSUM) as p2ps:
                    # v: [128 s, 256 w] per s-tile
                    for st in range(NQ):
                        psv = p2ps.tile([128, DPC], F32, tag="psv")
                        for dc in range(ND):
                            nc.tensor.matmul(
                                psv[:],
                                xT[dc][:, st * 128:(st + 1) * 128],
                                wv_sb[dc][:],
                                start=(dc == 0), stop=False)
                        nc.tensor.matmul(
                            psv[:], ones[0:1, 0:128],
                            bv_sb[:], start=False, stop=True)
                        nc.vector.tensor_copy(vv[st][:], psv[:])
                    # qT/kT per head: [64 w, 2048 s]
                    for h in range(HPC):
                        hsl = slice(h * W, (h + 1) * W)
                        for sl4 in range(4):
                            ssl = slice(sl4 * 512, (sl4 + 1) * 512)
                            psq = p2ps.tile([64, 512], F32, tag="psq")
                            psk = p2ps.tile([64, 512], F32, tag="psk")
                            for dc in range(ND):
                                nc.tensor.matmul(
                                    psq[:], wq_sb[dc][:, hsl], xT[dc][:, ssl],
                                    start=(dc == 0), stop=False)
                            nc.tensor.matmul(
                                psq[:], bq_sb[0:1, hsl], ones[0:1, 0:512],
                                start=False, stop=True)
                            nc.scalar.copy(qT[h][0:64, ssl], psq[:])
                            for dc in range(ND):
                                nc.tensor.matmul(
                                    psk[:], wk_sb[dc][:, hsl], xT[dc][:, ssl],
                                    start=(dc == 0), stop=False)
                            nc.tensor.matmul(
                                psk[:], bk_sb[0:1, hsl], ones[0:1, 0:512],
                                start=False, stop=True)
                            nc.scalar.copy(kT[h][0:64, ssl], psk[:])

            # ---- Phase 3: per head: pass A, ln-prep, pass B ----
            with (
                tc.tile_pool(name="pA", bufs=2, space=PSUM) as pA,
                tc.tile_pool(name="pB", bufs=1, space=PSUM) as pB,
                tc.tile_pool(name="pH", bufs=1, space=PSUM) as pH,
                tc.tile_pool(name="oA", bufs=3) as oA,
                tc.tile_pool(name="oB", bufs=3) as oB,
                tc.tile_pool(name="lnp", bufs=2) as lnp,
            ):
                psh_cur = [None]  # live hT accumulator tile

                def emit_A(h, i):
                    qsl = slice(i * 128, (i + 1) * 128)
                    scs = oA.tile([128, S], F32, tag="scs", name="scs")
                    ex = oA.tile([128, S], F32, tag="ex", name="ex")
                    for half in range(2):
                        ps = pA.tile([128, 1024], F32, tag="psA", name="psA")
                        for jj in range(2):
                            k0 = half * 1024 + jj * 512
                            nc.tensor.matmul(
                                ps[:, jj * 512:(jj + 1) * 512],
                                qT[h][0:65, qsl],
                                kT[h][0:65, k0:k0 + 512],
                                start=True, stop=True)
                        hs = slice(half * 1024, (half + 1) * 1024)
                        nc.vector.tensor_copy(scs[:, hs], ps[:])
                        nc.scalar.activation(
                            ex[:, hs], ps[:], AF.Exp,
                            accum_out=dn2[h][:, 2 * i + half:2 * i + half + 1])
                    nc.sync.dma_start(
                        sc_d.ap()[h * S + i * 128:h * S + (i + 1) * 128, :],
                        scs[:])
                    nc.vector.tensor_add(ds[h][:, i:i + 1],
                                         dn2[h][:, 2 * i:2 * i + 1],
                                         dn2[h][:, 2 * i + 1:2 * i + 2])
                    nc.vector.reciprocal(rc[h][:, i:i + 1], ds[h][:, i:i + 1])
                    sf = oA.tile([128, S], F32, tag="sf", name="sf")
                    nc.vector.tensor_scalar_mul(sf[:], ex[:], rc[h][:, i:i + 1])
                    nc.sync.dma_start(
                        sm_d.ap()[h * S + i * 128:h * S + (i + 1) * 128, :],
                        sf[:])

                def emit_lnprep(h):
                    # qT rows 65/66 = -ln(denom) split into fp32r hi+lo
                    lnq = lnp.tile([128, NQ], F32, tag="lnq", name="lnq")
                    nc.scalar.activation(lnq[:], rc[h][:], AF.Ln)
                    lnhi = lnp.tile([128, 2 * NQ], F32R, tag="lnhi", name="lnhi")
                    nc.vector.tensor_copy(lnhi[:, 0:NQ], lnq[:])
                    nc.vector.tensor_sub(lnhi[:, NQ:2 * NQ], lnq[:],
                                         lnhi[:, 0:NQ])
                    pst2 = pB.tile([2 * NQ, 128], F32R, tag="psb", name="pst2")
                    nc.tensor.transpose(pst2[:], lnhi[:], ident[:])
                    lnrow = lnp.tile([2 * NQ, 128], F32R, tag="lnrow",
                                     name="lnrow")
                    nc.vector.tensor_copy(lnrow[:], pst2[:])
                    nc.sync.dma_start(qT[h][65:66, :], lnrow[0:NQ, :])
                    nc.sync.dma_start(qT[h][66:67, :], lnrow[NQ:2 * NQ, :])

                def emit_B(h, qhalf, j):
                    hsl = slice(h * W, (h + 1) * W)
                    ksl = slice(j * 128, (j + 1) * 128)
                    if j == 0:
                        psh_cur[0] = pH.tile([64, 1024], F32, tag="psh",
                                             name="psh")
                    psh = psh_cur[0]
                    psb = pB.tile([128, 1024], F32, tag="psb", name="psb")
                    for jj in range(2):
                        q0 = qhalf * 1024 + jj * 512
                        nc.tensor.matmul(
                            psb[:, jj * 512:(jj + 1) * 512],
                            kT[h][0:67, ksl],
                            qT[h][0:67, q0:q0 + 512],
                            start=True, stop=True)
                    exb = oB.tile([128, 1024], F32R, tag="exb", name="exb")
                    nc.scalar.activation(exb[:], psb[:], AF.Exp)
                    for jj in range(2):
                        nc.tensor.matmul(
                            psh[:, jj * 512:(jj + 1) * 512],
                            vv[j][:, hsl],
                            exb[:, jj * 512:(jj + 1) * 512],
                            start=(j == 0), stop=(j == NQ - 1))
                    if j == NQ - 1:
                        hst = oB.tile([64, 1024], F32, tag="hst", name="hst")
                        nc.vector.tensor_copy(hst[:], psh[:])
                        nc.sync.dma_start(
                            hT_d.ap()[h * W:(h + 1) * W,
                                      qhalf * 1024:(qhalf + 1) * 1024],
                            hst[:])

                # Software pipeline: pass B runs one head behind pass A so
                # B matmuls fill the PE gaps in A's copy/exp-limited stretches.
                for h in range(HPC):
                    for i in range(NQ):
                        emit_A(h, i)
                        if h > 0:
                            for u in range(2):
                                unit = i * 2 + u
                                qhalf, j = divmod(unit, NQ)
                                emit_B(h - 1, qhalf, j)
                    emit_lnprep(h)
                for unit in range(2 * NQ):
                    qhalf, j = divmod(unit, NQ)
                    emit_B(HPC - 1, qhalf, j)

    nc.compile()
    return nc


def kernel(**inputs) -> tuple:
    global _compiled, last_results
    from concourse import bass_utils

    x = np.asarray(inputs["x"], np.float32)
    mask = np.asarray(inputs["mask"], np.float32)
    wq = np.asarray(inputs["wq"], np.float32)
    bq = np.asarray(inputs["bq"], np.float32)
    wk = np.asarray(inputs["wk"], np.float32)
    bk = np.asarray(inputs["bk"], np.float32)
    wv = np.asarray(inputs["wv"], np.float32)
    bv = np.asarray(inputs["bv"], np.float32)

    if _compiled is None:
        _compiled = _build_program()
    nc = _compiled

    ident = np.eye(128, dtype=np.float32)
    in_maps = []
    for c in range(N_CORES):
        b, hg = divmod(c, HPC)
        dsl = slice(hg * DPC, (hg + 1) * DPC)
        in_maps.append({
            "x": np.ascontiguousarray(x[b]),
            "wq": np.ascontiguousarray(wq[:, dsl]) * 0.125,
            "wk": np.ascontiguousarray(wk[:, dsl]),
            "wv": np.ascontiguousarray(wv[:, dsl]),
            "bq": (bq[dsl] * 0.125).reshape(1, DPC).astype(np.float32),
            "bk": bk[dsl].reshape(1, DPC).astype(np.float32),
            "bv": bv[dsl].reshape(1, DPC).astype(np.float32),
            "mk": (-10000.0 * (1.0 - mask[b])).reshape(1, S).astype(np.float32),
            "onesrow": np.ones((1, S), np.float32),
            "ident": ident,
        })

    res = bass_utils.run_bass_kernel_spmd(
        nc, in_maps, core_ids=list(range(N_CORES)),
        trace=bool(os.environ.get("KERNEL_TRACE")))
    last_results = res

    h_full = np.empty((B, S, D), np.float32)
    scores = np.empty((B, H, S, S), np.float32)
    soft = np.empty((B, H, S, S), np.float32)
    for c in range(N_CORES):
        b, hg = divmod(c, HPC)
        r = res.results[c]
        scores[b, hg * HPC:(hg + 1) * HPC] = r["scores"].reshape(HPC, S, S)
        soft[b, hg * HPC:(hg + 1) * HPC] = r["soft"].reshape(HPC, S, S)
        h_full[b, :, hg * DPC:(hg + 1) * DPC] = r["houtT"].T
    return h_full, soft, scores


# revision 25
# speedup vs baseline: 3.3830x; 1.1010x over previous
"""Multi-head attention (B=2, S=2048, D=1024, H=16) on 8 Trainium2 NeuronCores.

Sharding: data-parallel over batch (2) x tensor-parallel over head groups (4),
so each of the 8 cores owns 4 heads of one batch element: its own slice of the
QKV projections, its own 4x(2048x2048) score slabs, softmax, and attention
output columns. All outputs are disjoint -> no collectives.

Kernel structure per core (all matmuls fp32r = full-rate ~tf32):
  1. PE-transpose x (2048x1024) -> xT chunks [128 d, 2048 s].
  2. Projections: qT/kT per head as [67, 2048] (head width on partitions).
     Aug rows fold affine terms into the score matmuls:
       row 64: qT=1, kT=-1e4*(1-mask)  -> additive mask
       row 65/66: qT=-ln(denom) hi/lo, kT=1 -> softmax normalization (pass B
       only; written after pass A computes the denominators)
     wq is pre-scaled by 1/8 host-side so score PSUM = final scores.
     v as [128 s, 256 w] tiles (natural layout for the AV contraction).
  3. Per head, pass A (q-major, contraction rows 0..64): scores -> PSUM
     [128,1024] halves; DVE copies scores out; ACT Exp with fused per-row
     accum -> denominators; DVE reciprocal + normalize -> scores_soft.
  4. ln-prep: ACT Ln(1/denom) -> hi/lo fp32r split -> PE transpose ->
     DMA into qT rows 65/66.
  5. Pass B (k-major, contraction rows 0..66): scoresT+mask-ln(denom) ->
     ACT Exp (= normalized weights) -> AV matmuls accumulate hT [64,1024]
     per q-half -> copy out -> DMA. h is emitted transposed (DPC, S);
     the host transposes back.
"""

import os
import sys

if "/opt/trn_rl_repo" not in sys.path:
    sys.path.insert(0, "/opt/trn_rl_repo")

import numpy as np

B, S, D, H = 2, 2048, 1024, 16
W = 64            # head width
N_CORES = 8
HPC = 4           # heads per core
DPC = HPC * W     # 256 projection columns per core
NQ = S // 128     # 16 row tiles
ND = D // 128     # 8 contraction chunks

_compiled = None
last_results = None  # stashed BassKernelResults for test harnesses


def _build_program():
    import concourse.bass as bass
    import concourse.bacc as bacc
    import concourse.tile as tile
    import concourse.mybir as mybir

    F32R = mybir.dt.float32r
    F32 = mybir.dt.float32
    AF = mybir.ActivationFunctionType
    PSUM = bass.MemorySpace.PSUM

    nc = bacc.Bacc("TRN2", target_bir_lowering=False, debug=False,
                   num_devices=N_CORES)

    x_d = nc.dram_tensor("xt", (D, S), F32R, kind="ExternalInput")
    wq_d = nc.dram_tensor("wq", (D, DPC), F32R, kind="ExternalInput")
    wk_d = nc.dram_tensor("wk", (D, DPC), F32R, kind="ExternalInput")
    wv_d = nc.dram_tensor("wv", (D, DPC), F32R, kind="ExternalInput")
    bq_d = nc.dram_tensor("bq", (1, DPC), F32R, kind="ExternalInput")
    bk_d = nc.dram_tensor("bk", (1, DPC), F32R, kind="ExternalInput")
    bv_d = nc.dram_tensor("bv", (1, DPC), F32R, kind="ExternalInput")
    mk_d = nc.dram_tensor("mk", (1, S), F32R, kind="ExternalInput")
    on_d = nc.dram_tensor("onesrow", (1, S), F32R, kind="ExternalInput")
    id_d = nc.dram_tensor("ident", (128, 128), F32R, kind="ExternalInput")

    sc_d = nc.dram_tensor("scores", (HPC * S, S), F32, kind="ExternalOutput")
    sm_d = nc.dram_tensor("soft", (HPC * S, S), F32, kind="ExternalOutput")
    hT_d = nc.dram_tensor("houtT", (DPC, S), F32, kind="ExternalOutput")

    with tile.TileContext(nc) as tc:
        with (
            tc.tile_pool(name="consts", bufs=1) as cp,
            tc.tile_pool(name="persist", bufs=1) as pp,
        ):
            ident = cp.tile([128, 128], F32R, tag="ident")
            nc.sync.dma_start(ident[:], id_d.ap()[:])
            ones = cp.tile([1, 512], F32R, tag="ones")
            nc.sync.dma_start(ones[:], on_d.ap()[0:1, 0:512])
            bq_sb = cp.tile([1, DPC], F32R, tag="bq")
            bk_sb = cp.tile([1, DPC], F32R, tag="bk")
            bv_sb = cp.tile([1, DPC], F32R, tag="bv")
            nc.sync.dma_start(bq_sb[:], bq_d.ap()[:])
            nc.sync.dma_start(bk_sb[:], bk_d.ap()[:])
            nc.sync.dma_start(bv_sb[:], bv_d.ap()[:])

            qT = [pp.tile([67, S], F32R, tag=f"qT{h}", name=f"qT{h}")
                  for h in range(HPC)]
            kT = [pp.tile([67, S], F32R, tag=f"kT{h}", name=f"kT{h}")
                  for h in range(HPC)]
            vv = [pp.tile([128, DPC], F32R, tag=f"v{st}", name=f"v{st}")
                  for st in range(NQ)]
            # per-head softmax stats: dn2 = per-half-row sums, ds = full sums,
            # rc = reciprocals
            dn2 = [pp.tile([128, 2 * NQ], F32, tag=f"dn{h}", name=f"dn{h}")
                   for h in range(HPC)]
            ds = [pp.tile([128, NQ], F32, tag=f"ds{h}", name=f"ds{h}")
                  for h in range(HPC)]
            rc = [pp.tile([128, NQ], F32, tag=f"rc{h}", name=f"rc{h}")
                  for h in range(HPC)]
            for h in range(HPC):
                nc.gpsimd.dma_start(qT[h][64:65, :], on_d.ap()[:])
                nc.gpsimd.dma_start(kT[h][64:65, :], mk_d.ap()[:])
                nc.gpsimd.dma_start(kT[h][65:66, :], on_d.ap()[:])
                nc.gpsimd.dma_start(kT[h][66:67, :], on_d.ap()[:])

            # One PSUM pool shared by every phase (no pool-transition
            # barriers): tags psA (2x2 banks), psb (2), psh (2) = 8 banks.
            with tc.tile_pool(name="mps", bufs=1, space=PSUM) as mps:
              with (
                tc.tile_pool(name="xt", bufs=1) as xtp,
                tc.tile_pool(name="wts", bufs=1) as wtp,
              ):
                xT = [xtp.tile([128, S], F32R, tag=f"xT{dc}", name=f"xT{dc}")
                      for dc in range(ND)]
                wq_sb = [wtp.tile([128, DPC], F32R, tag=f"wq{dc}", name=f"wq{dc}")
                         for dc in range(ND)]
                wk_sb = [wtp.tile([128, DPC], F32R, tag=f"wk{dc}", name=f"wk{dc}")
                         for dc in range(ND)]
                wv_sb = [wtp.tile([128, DPC], F32R, tag=f"wv{dc}", name=f"wv{dc}")
                         for dc in range(ND)]
                for dc in range(ND):
                    sl = slice(dc * 128, (dc + 1) * 128)
                    nc.sync.dma_start(
                        xT[dc][:], x_d.ap()[sl, :])
                    nc.scalar.dma_start(wq_sb[dc][:], wq_d.ap()[sl, :])
                    nc.scalar.dma_start(wk_sb[dc][:], wk_d.ap()[sl, :])
                    nc.scalar.dma_start(wv_sb[dc][:], wv_d.ap()[sl, :])

                def emit_qk_stripe(sl4):
                    ssl = slice(sl4 * 512, (sl4 + 1) * 512)
                    for hp in range(2):
                        psl = slice(hp * 128, (hp + 1) * 128)
                        psq = mps.tile([128, 512], F32, tag="psA", name="psq",
                                       bufs=2)
                        for dc in range(ND):
                            nc.tensor.matmul(
                                psq[:], wq_sb[dc][:, psl], xT[dc][:, ssl],
                                start=(dc == 0), stop=False)
                        nc.tensor.matmul(
                            psq[:], bq_sb[0:1, psl], ones[0:1, 0:512],
                            start=False, stop=True)
                        nc.scalar.copy(qT[2 * hp][0:64, ssl], psq[0:64, :])
                        nc.scalar.copy(qT[2 * hp + 1][0:64, ssl],
                                       psq[64:128, :])
                        psk = mps.tile([128, 512], F32, tag="psA", name="psk",
                                       bufs=2)
                        for dc in range(ND):
                            nc.tensor.matmul(
                                psk[:], wk_sb[dc][:, psl], xT[dc][:, ssl],
                                start=(dc == 0), stop=False)
                        nc.tensor.matmul(
                            psk[:], bk_sb[0:1, psl], ones[0:1, 0:512],
                            start=False, stop=True)
                        nc.vector.tensor_copy(kT[2 * hp][0:64, ssl],
                                              psk[0:64, :])
                        nc.vector.tensor_copy(kT[2 * hp + 1][0:64, ssl],
                                              psk[64:128, :])

                def emit_v(st):
                    psv = mps.tile([128, DPC], F32,
                                   tag=("psb" if st % 2 else "psh"),
                                   name="psv")
                    for dc in range(ND):
                        nc.tensor.matmul(
                            psv[:], xT[dc][:, st * 128:(st + 1) * 128],
                            wv_sb[dc][:], start=(dc == 0), stop=False)
                    nc.tensor.matmul(
                        psv[:], ones[0:1, 0:128], bv_sb[:],
                        start=False, stop=True)
                    nc.vector.tensor_copy(vv[st][:], psv[:])

                for sl4 in range(4):
                    emit_qk_stripe(sl4)
                for st in range(NQ):
                    emit_v(st)

              # ---- Phase 3: per head: pass A, ln-prep, pass B ----
              with (
                tc.tile_pool(name="oA", bufs=2) as oA,
                tc.tile_pool(name="oB", bufs=3) as oB,
                tc.tile_pool(name="lnp", bufs=2) as lnp,
              ):
                pA = pB = pH = mps  # same pool, tags select the banks
                psh_cur = [None]  # live hT accumulator tile

                def emit_A(h, i):
                    qsl = slice(i * 128, (i + 1) * 128)
                    scs = oA.tile([128, S], F32, tag="scs", name="scs")
                    ex = oA.tile([128, S], F32, tag="ex", name="ex")
                    for half in range(2):
                        ps = pA.tile([128, 1024], F32, tag="psA", name="psA",
                                     bufs=2)
                        for jj in range(2):
                            k0 = half * 1024 + jj * 512
                            nc.tensor.matmul(
                                ps[:, jj * 512:(jj + 1) * 512],
                                qT[h][0:65, qsl],
                                kT[h][0:65, k0:k0 + 512],
                                start=True, stop=True)
                        hs = slice(half * 1024, (half + 1) * 1024)
                        nc.vector.tensor_copy(scs[:, hs], ps[:])
                        nc.scalar.activation(
                            ex[:, hs], ps[:], AF.Exp,
                            accum_out=dn2[h][:, 2 * i + half:2 * i + half + 1])
                    nc.sync.dma_start(
                        sc_d.ap()[h * S + i * 128:h * S + (i + 1) * 128, :],
                        scs[:])
                    nc.vector.tensor_add(ds[h][:, i:i + 1],
                                         dn2[h][:, 2 * i:2 * i + 1],
                                         dn2[h][:, 2 * i + 1:2 * i + 2])
                    nc.vector.reciprocal(rc[h][:, i:i + 1], ds[h][:, i:i + 1])
                    nc.vector.tensor_scalar_mul(ex[:], ex[:], rc[h][:, i:i + 1])
                    nc.sync.dma_start(
                        sm_d.ap()[h * S + i * 128:h * S + (i + 1) * 128, :],
                        ex[:])

                def emit_lnprep(h):
                    # qT rows 65/66 = -ln(denom) split into fp32r hi+lo
                    lnq = lnp.tile([128, NQ], F32, tag="lnq", name="lnq")
                    nc.scalar.activation(lnq[:], rc[h][:], AF.Ln)
                    lnhi = lnp.tile([128, 2 * NQ], F32R, tag="lnhi", name="lnhi")
                    nc.vector.tensor_copy(lnhi[:, 0:NQ], lnq[:])
                    nc.vector.tensor_sub(lnhi[:, NQ:2 * NQ], lnq[:],
                                         lnhi[:, 0:NQ])
                    pst2 = pB.tile([2 * NQ, 128], F32R, tag="psb", name="pst2")
                    nc.tensor.transpose(pst2[:], lnhi[:], ident[:])
                    lnrow = lnp.tile([2 * NQ, 128], F32R, tag="lnrow",
                                     name="lnrow")
                    nc.vector.tensor_copy(lnrow[:], pst2[:])
                    nc.sync.dma_start(qT[h][65:66, :], lnrow[0:NQ, :])
                    nc.sync.dma_start(qT[h][66:67, :], lnrow[NQ:2 * NQ, :])

                def emit_B(h, qhalf, j):
                    hsl = slice(h * W, (h + 1) * W)
                    ksl = slice(j * 128, (j + 1) * 128)
                    if j == 0:
                        psh_cur[0] = pH.tile([64, 1024], F32, tag="psh",
                                             name="psh")
                    psh = psh_cur[0]
                    psb = pB.tile([128, 1024], F32, tag="psb", name="psb")
                    for jj in range(2):
                        q0 = qhalf * 1024 + jj * 512
                        nc.tensor.matmul(
                            psb[:, jj * 512:(jj + 1) * 512],
                            kT[h][0:67, ksl],
                            qT[h][0:67, q0:q0 + 512],
                            start=True, stop=True)
                    exb = oB.tile([128, 1024], F32R, tag="exb", name="exb")
                    nc.scalar.activation(exb[:], psb[:], AF.Exp)
                    for jj in range(2):
                        nc.tensor.matmul(
                            psh[:, jj * 512:(jj + 1) * 512],
                            vv[j][:, hsl],
                            exb[:, jj * 512:(jj + 1) * 512],
                            start=(j == 0), stop=(j == NQ - 1))
                    if j == NQ - 1:
                        hst = oB.tile([64, 1024], F32, tag="hst", name="hst")
                        nc.vector.tensor_copy(hst[:], psh[:])
                        nc.sync.dma_start(
                            hT_d.ap()[h * W:(h + 1) * W,
                                      qhalf * 1024:(qhalf + 1) * 1024],
                            hst[:])

                # Software pipeline: pass B runs one head behind pass A so
                # B matmuls fill the PE gaps in A's copy/exp-limited stretches.
                LAG = 4  # B units trail A tiles so B spans head boundaries
                for h in range(HPC):
                    for i in range(NQ):
                        emit_A(h, i)
                        if h > 0:
                            for unit in (i * 2 - LAG, i * 2 - LAG + 1):
                                if unit >= 0:
                                    emit_B(h - 1, *divmod(unit, NQ))
                    if h > 0:
                        for unit in range(2 * NQ - LAG, 2 * NQ):
                            emit_B(h - 1, *divmod(unit, NQ))
                    emit_lnprep(h)
                for unit in range(2 * NQ):
                    emit_B(HPC - 1, *divmod(unit, NQ))

    nc.compile()
    return nc


def kernel(**inputs) -> tuple:
    global _compiled, last_results
    from concourse import bass_utils

    x = np.asarray(inputs["x"], np.float32)
    mask = np.asarray(inputs["mask"], np.float32)
    wq = np.asarray(inputs["wq"], np.float32)
    bq = np.asarray(inputs["bq"], np.float32)
    wk = np.asarray(inputs["wk"], np.float32)
    bk = np.asarray(inputs["bk"], np.float32)
    wv = np.asarray(inputs["wv"], np.float32)
    bv = np.asarray(inputs["bv"], np.float32)

    if _compiled is None:
        _compiled = _build_program()
    nc = _compiled

    ident = np.eye(128, dtype=np.float32)
    in_maps = []
    for c in range(N_CORES):
        b, hg = divmod(c, HPC)
        dsl = slice(hg * DPC, (hg + 1) * DPC)
        in_maps.append({
            "xt": np.ascontiguousarray(x[b].T),
            "wq": np.ascontiguousarray(wq[:, dsl]) * 0.125,
            "wk": np.ascontiguousarray(wk[:, dsl]),
            "wv": np.ascontiguousarray(wv[:, dsl]),
            "bq": (bq[dsl] * 0.125).reshape(1, DPC).astype(np.float32),
            "bk": bk[dsl].reshape(1, DPC).astype(np.float32),
            "bv": bv[dsl].reshape(1, DPC).astype(np.float32),
            "mk": (-10000.0 * (1.0 - mask[b])).reshape(1, S).astype(np.float32),
            "onesrow": np.ones((1, S), np.float32),
            "ident": ident,
        })

    res = bass_utils.run_bass_kernel_spmd(
        nc, in_maps, core_ids=list(range(N_CORES)),
        trace=bool(os.environ.get("KERNEL_TRACE")))
    last_results = res

    h_full = np.empty((B, S, D), np.float32)
    scores = np.empty((B, H, S, S), np.float32)
    soft = np.empty((B, H, S, S), np.float32)
    for c in range(N_CORES):
        b, hg = divmod(c, HPC)
        r = res.results[c]
        scores[b, hg * HPC:(hg + 1) * HPC] = r["scores"].reshape(HPC, S, S)
        soft[b, hg * HPC:(hg + 1) * HPC] = r["soft"].reshape(HPC, S, S)
        h_full[b, :, hg * DPC:(hg + 1) * DPC] = r["houtT"].T
    return h_full, soft, scores
